# revision 29
# baseline (speedup 1.0000x reference)
"""Trainium2 Bass kernel for nn_D_GA_1812476199112 (maxpool -> 16-head
attention over 1024 tokens -> proj -> batchnorm -> maxunpool).

Sharding: data-parallel over batch B=8, one batch element per NeuronCore.
Everything is local per core; no collectives.

Per-core pipeline, v2 ("flip-AV + 2-engine exp"):
  * Pre/post stages run in a [128, 2048] partition-packed layout
    (partition = channel + 64*image-half), halving the per-op free sizes
    of maxpool / argmax masks / unpool and the x/out DMA times. All of
    them run on GPSIMD (which cannot touch PSUM), freeing ACT/DVE.
  * QKV prep matmuls run in f32r (1 cycle/row) with strip-packed
    host-permuted weights; Q/K packs are evicted PSUM->SBUF as paired
    [128, 1024] copies alternating ACT/DVE.
  * Scores S^T [128 keys, 1024 queries] per (head, kt) chunk (two 512-q
    matmuls into one 2-bank PSUM slot, 3 slots); exp is split across ACT
    (table exp, bf16 out) and DVE (Schraudolph bit-trick: int16
    y = round(a*s + b) reinterpreted as bf16 ~ exp(s/2), ~3% elementwise
    error that largely cancels in the softmax ratio; ~1e-3 end-to-end).
  * AV is "flipped": each E block [128k, 128q] is the stationary operand
    (weight load is free) against a ones-augmented V~ [128k, 5] moving
    operand, producing o^T [128q, 5] in 5 cycles/matmul, accumulated over
    key tiles into a memset 2-bank PSUM accumulator with start=False
    (a start=True would mark the whole 2KB zero-region pending-zero and
    wipe sibling regions); denominators ride along in column 4.
  * Tail: strided DVE reciprocal+normalize in [q, (h,d)] layout, identity
    matmul transpose back to [(h,d), q], f32r proj, fused BN via ACT
    Identity(scale,bias), unpool via masked multiplies on GPSIMD.
"""
import numpy as np

DIM = 64
HEAD_DIM = 4
NUM_HEADS = 16
B = 8
H = W = 64
HP = WP = 32
N = HP * WP          # 1024 tokens
NKT = 8              # key tiles of 128
BN_EPS = 1e-5

# Schraudolph constants for exp(0.5*s) in bf16-bits:
#   int16 y = round(A_SCH * s + B_SCH); reinterpret(y) ~ exp(0.5*s)
A_SCH = 0.5 * 128.0 / float(np.log(2.0))
B_SCH = 127.0 * 128.0 - 5.5

# wb column layout
_WQP0 = 0              # 4 x [128, 128] strip-packed q weights (doubled rows)
_WKP0 = 512            # 4 x [128, 128] strip-packed k weights
_WV0 = 1024            # [128, 64] v weights (doubled rows)
_WPJ0 = 1088           # [64, 64] proj weights (rows 0..64)
_I0 = 1152             # [128, 128] identity
_BNS = 1280            # [128, 1] bn scale (doubled)
_BNB = 1281            # [128, 1] bn bias (doubled)
_WBC = 1282

_CACHE = {}


def _region_col(m):
    """PSUM accumulator column for region m = 8*head + qtile (5 cols each,
    split so no [*,5] block crosses the 512-col PSUM bank boundary)."""
    return 5 * m if m < 102 else 512 + 5 * (m - 102)


def _exp_schedule():
    """Strict weighted interleave of the 128 exp chunks across ACT/DVE
    (Bresenham at the inverse-cost ratio) so the two engines always run
    concurrently; runs of the same engine serialize the 3-slot pipeline.
    The first few chunks go to ACT while DVE finishes its prep copies."""
    n_act_lead = 2
    frac_act = 0.5   # exact alternation avoids periodic double-ACT hiccups
    out = ["act"] * n_act_lead
    acc = 0.0
    for _ in range(128 - n_act_lead):
        acc += frac_act
        if acc >= 1.0:
            out.append("act")
            acc -= 1.0
        else:
            out.append("dve")
    return out


def _build_program():
    import concourse.mybir as mybir
    import concourse.tile as tile
    from concourse import bacc

    f32 = mybir.dt.float32
    f32r = mybir.dt.float32r
    bf16 = mybir.dt.bfloat16
    i16 = mybir.dt.int16
    AF = mybir.ActivationFunctionType
    OP = mybir.AluOpType

    nc = bacc.Bacc("TRN2", debug=False)

    x_d = nc.dram_tensor("x", [DIM, H * W], f32, kind="ExternalInput").ap()
    wb_d = nc.dram_tensor("wb", [128, _WBC], f32r, kind="ExternalInput").ap()
    out_d = nc.dram_tensor("out", [DIM, H * W], f32, kind="ExternalOutput").ap()

    with tile.TileContext(nc) as tc:
        with (
            tc.tile_pool(name="singles", bufs=1) as sg1,
            tc.tile_pool(name="expp", bufs=6) as expp,
            tc.tile_pool(name="spsum", bufs=3, space="PSUM") as spsum,
            tc.tile_pool(name="opsum", bufs=1, space="PSUM") as opsum,
        ):
            # hoist the ACT exp-table load to t=0 via a dummy exp
            warm = sg1.tile([1, 1], f32)
            nc.vector.memset(warm, 0.0)
            nc.scalar.activation(warm, warm, AF.Exp)
            ib16 = sg1.tile([128, 128], bf16)

            # ---------- input DMAs ----------
            x_sb = sg1.tile([128, H * W // 2], f32)
            nc.sync.dma_start(out=x_sb[0:64, :], in_=x_d[:, 0:2048])
            nc.sync.dma_start(out=x_sb[64:128, :], in_=x_d[:, 2048:4096])
            wb_sb = sg1.tile([128, _WBC], f32r)
            nc.sync.dma_start(out=wb_sb[:, 0:1024], in_=wb_d[:, 0:1024])
            nc.sync.dma_start(out=wb_sb[:, 1024:_WBC], in_=wb_d[:, 1024:_WBC])
            wqp = [wb_sb[:, _WQP0 + 128 * s:_WQP0 + 128 * s + 128] for s in range(4)]
            wkp = [wb_sb[:, _WKP0 + 128 * s:_WKP0 + 128 * s + 128] for s in range(4)]
            wv_sb = wb_sb[:, _WV0:_WV0 + 64]
            wpj_sb = wb_sb[:, _WPJ0:_WPJ0 + 64]
            i128_sb = wb_sb[:, _I0:_I0 + 128]
            bns_sb = wb_sb[:, _BNS:_BNS + 1].bitcast(f32)
            bnb_sb = wb_sb[:, _BNB:_BNB + 1].bitcast(f32)

            # ---------- PE warmup (p-state ramp) ----------
            dummy_bf = sg1.tile([64, 512], bf16)
            nc.vector.memset(dummy_bf, 1.0)
            for _ in range(11):
                w_ps = spsum.tile([128, 1024], f32, tag="slot")
                nc.tensor.matmul(w_ps[:, 0:512], dummy_bf[:, 0:128], dummy_bf,
                                 start=True, stop=True)

            # ---------- maxpool (DVE max; GPSIMD lacks max/is_equal) ----------
            # x_sb per-partition layout: 32 h-rows x 64 w; window elems:
            xr = x_sb.rearrange("p (i ti j tj) -> p i ti j tj", ti=2, tj=2, j=WP)
            v = [xr[:, :, 0, :, 0], xr[:, :, 0, :, 1],
                 xr[:, :, 1, :, 0], xr[:, :, 1, :, 1]]
            m01 = sg1.tile([128, N // 2], f32)
            m23 = sg1.tile([128, N // 2], f32)
            m01r = m01.rearrange("p (i j) -> p i j", j=WP)
            m23r = m23.rearrange("p (i j) -> p i j", j=WP)
            pooled = sg1.tile([128, N // 2], f32)
            pooledfr = pooled.rearrange("p (i j) -> p i j", j=WP)
            pooled2 = sg1.tile([128, N // 2], f32r)
            pooled2r = pooled2.rearrange("p (i j) -> p i j", j=WP)
            nc.vector.tensor_tensor(m01r, v[0], v[1], op=OP.max)
            nc.vector.tensor_tensor(m23r, v[2], v[3], op=OP.max)
            # f32r first (gates prep); exact f32 after (gates only masks)
            nc.vector.tensor_tensor(pooled2r, m01r, m23r, op=OP.max)
            nc.vector.tensor_tensor(pooledfr, m01r, m23r, op=OP.max)

            # ---------- argmax masks, entirely on idle GPSIMD ----------
            # eq via saturating int16 trick: d = pooled - v >= 0;
            # int16(-3.4e38*d + 16256) = 16256 (bf16 1.0) iff d == 0, else
            # saturates to -32768 (bf16 -0.0). First-match chain as usual.
            masks = []
            nf = None
            for p in range(4):
                dterm = sg1.tile([128, N // 2], f32, tag=f"dt{p}")
                nc.gpsimd.tensor_tensor(
                    dterm.rearrange("p (i j) -> p i j", j=WP), pooledfr, v[p],
                    op=OP.subtract)
                eqi = sg1.tile([128, N // 2], i16, tag=f"eqi{p}")
                nc.gpsimd.tensor_scalar(eqi, dterm, -3.4e38, 16256.0,
                                        op0=OP.mult, op1=OP.add)
                eq = eqi[:, :].bitcast(bf16)
                if p == 0:
                    nf = sg1.tile([128, N // 2], f32, tag="nf0")
                    nc.gpsimd.tensor_scalar(nf, eq, -1.0, 1.0,
                                            op0=OP.mult, op1=OP.add)
                    mk0 = sg1.tile([128, N // 2], f32, tag="mk0")
                    nc.gpsimd.tensor_scalar(mk0, nf, -1.0, 1.0,
                                            op0=OP.mult, op1=OP.add)
                    masks.append(mk0[:, :])
                else:
                    mk = sg1.tile([128, N // 2], f32, tag=f"mk{p}")
                    nc.gpsimd.tensor_tensor(mk, eq, nf, op=OP.mult)
                    masks.append(mk[:, :])
                    if p < 3:
                        nf2 = sg1.tile([128, N // 2], f32, tag=f"nf{p}")
                        nc.gpsimd.tensor_tensor(nf2, nf, mk, op=OP.subtract)
                        nf = nf2

            # ---------- QKV prep (f32r matmuls; paired ACT/DVE evictions) ----
            qtp = [sg1.tile([128, N], f32r, tag=f"qtp{s}", name=f"qtp{s}")
                   for s in range(4)]
            ktp = [sg1.tile([128, N], f32r, tag=f"ktp{s}", name=f"ktp{s}")
                   for s in range(4)]
            pooled_r = pooled2[:, :]

            def emit_prep(s):
                # sg0's q-pack eviction goes to DVE (parallel with ACT's k0
                # at startup); later sgs' evictions run mid-attention and go
                # to ACT, which has slack (DVE is the bottleneck engine).
                for w_sb, pack, eng in ((wkp[s], ktp[s], nc.scalar.copy),
                                        (wqp[s], qtp[s],
                                         nc.vector.tensor_copy if s == 0
                                         else nc.scalar.copy)):
                    p_ps = spsum.tile([128, 1024], f32, tag="slot",
                                      name="p_ps")
                    for h in range(2):
                        psl = slice(64 * h, 64 * h + 64)
                        nc.tensor.matmul(p_ps[:, 512 * h:512 * h + 512],
                                         w_sb[psl],
                                         pooled_r[psl], start=True, stop=True)
                    eng(pack[:, :], p_ps)

            for s in range(4):
                emit_prep(s)

            # V~ tiles [128 keys, 16 heads * (4 dims + ones)] in bf16;
            # the per-kt prep matmul+copy is emitted inside the chunk loop
            # (at sg==0) so it doesn't delay the first score chunks.
            vt = [sg1.tile([128, 80], bf16, tag=f"vt{kt}", name=f"vt{kt}")
                  for kt in range(NKT)]
            for kt in range(NKT):
                nc.gpsimd.memset(vt[kt], 1.0)

            def emit_vprep(kt):
                h = kt // 4
                psl = slice(64 * h, 64 * h + 64)
                v_ps = spsum.tile([128, 1024], f32, tag="slot", name="v_ps")
                nc.tensor.matmul(
                    v_ps[:, 0:64],
                    pooled_r[psl, 128 * (kt % 4):128 * (kt % 4) + 128],
                    wv_sb[psl], start=True, stop=True)
                vcp = nc.scalar.copy if kt % 2 == 0 else nc.vector.tensor_copy
                vcp(vt[kt].rearrange("p (h e) -> p h e", e=5)[:, :, 0:4],
                    v_ps[:, 0:64].rearrange("p (h e) -> p h e", e=4))

            # ---------- attention ----------
            # Bank-init trick: one start=True matmul into each bank's pad
            # columns marks the whole 2KB zero-region pending-zero, so every
            # region's first start=False AV write initializes (no memset).
            # More than one start=True per bank would wipe sibling regions.
            o_acc = opsum.tile([128, 1024], f32)
            for bank_pad in (510, 1022):
                nc.tensor.matmul(o_acc[:, bank_pad:bank_pad + 2],
                                 dummy_bf[:, 0:128], dummy_bf[:, 0:2],
                                 start=True, stop=True,
                                 skip_group_check=True)
            assign = _exp_schedule()
            pend = []
            LAG = 3

            def flush_one():
                e_t, sg, c, kt = pend.pop(0)
                head = 4 * sg + c
                for J in range(8):
                    col = _region_col(16 * J + head)
                    nc.tensor.matmul(
                        o_acc[:, col:col + 5],
                        e_t[:, 128 * J:128 * J + 128].bitcast(bf16),
                        vt[kt][:, 5 * head:5 * head + 5],
                        start=False, stop=(kt == NKT - 1),
                        skip_group_check=True)

            ci = 0
            for sg in range(4):
                for kt in range(NKT):
                    for c in range(4):
                        s_ps = spsum.tile([128, 1024], f32, tag="slot")
                        for qh in range(2):
                            nc.tensor.matmul(
                                s_ps[:, 512 * qh:512 * qh + 512],
                                ktp[sg][32 * c:32 * c + 4,
                                        128 * kt:128 * kt + 128],
                                qtp[sg][32 * c:32 * c + 4,
                                        512 * qh:512 * qh + 512],
                                start=True, stop=True,
                                tile_position=(32 * c, 0))
                        e_t = expp.tile([128, 1024], i16, tag="exp")
                        eng = assign[ci]
                        ci += 1
                        if eng == "act":
                            nc.scalar.activation(
                                e_t[:, :].bitcast(bf16), s_ps,
                                AF.Exp, scale=0.5)
                        else:
                            nc.vector.tensor_scalar(
                                e_t, s_ps, A_SCH, B_SCH,
                                op0=OP.mult, op1=OP.add)
                        pend.append((e_t, sg, c, kt))
                        if ci <= NKT:
                            # spread the V~ preps over the first chunks
                            emit_vprep(ci - 1)
                        if len(pend) > LAG:
                            flush_one()
            while pend:
                flush_one()

            # ---------- tail ----------
            # denominators: region m col +4, strided over the 5-col regions
            o5a = o_acc[:, 0:510].rearrange("p (m e) -> p m e", e=5)
            o5b = o_acc[:, 512:642].rearrange("p (m e) -> p m e", e=5)
            rall = sg1.tile([128, 128], f32)
            nc.vector.reciprocal(rall[:, 0:102], o5a[:, :, 4])
            nc.vector.reciprocal(rall[:, 102:128], o5b[:, :, 4])
            rep = sg1.tile([128, 512], f32)
            rep4 = rep.rearrange("p (m e) -> p m e", e=4)
            for d in range(4):
                nc.vector.tensor_copy(rep4[:, :, d], rall)
            eno = sg1.tile([128, 512], bf16)
            eno4 = eno.rearrange("p (m e) -> p m e", e=4)
            nc.vector.tensor_tensor(eno4[:, 0:102], o5a[:, :, 0:4],
                                    rep4[:, 0:102], op=OP.mult)
            nc.vector.tensor_tensor(eno4[:, 102:128], o5b[:, :, 0:4],
                                    rep4[:, 102:128], op=OP.mult)
            # transpose back to [(h,d), q] via identity matmuls; eno cols are
            # (J, h, d)-contiguous under the J-major region map m = 16J+head
            nc.scalar.copy(ib16, i128_sb)
            ot_sb = sg1.tile([64, N], f32r)
            for jg in range(2):
                ot_ps = spsum.tile([128, 1024], f32, tag="slot")
                for jj in range(4):
                    J = 4 * jg + jj
                    nc.tensor.matmul(
                        ot_ps[0:64, 128 * jj:128 * jj + 128],
                        eno[:, 64 * J:64 * J + 64],
                        ib16,
                        start=True, stop=True)
                otcp = nc.scalar.copy if jg == 0 else nc.vector.tensor_copy
                otcp(ot_sb[:, 512 * jg:512 * jg + 512], ot_ps[0:64, 0:512])
            # proj + BN (per image-half) + unpool
            y128 = sg1.tile([128, 512], f32)
            out_sb = sg1.tile([128, H * W // 2], f32)
            outr = out_sb.rearrange("p (i ti j tj) -> p i ti j tj",
                                    ti=2, tj=2, j=WP)
            yr = y128.rearrange("p (i j) -> p i j", j=WP)
            for h in range(2):
                psl = slice(64 * h, 64 * h + 64)
                pj_ps = spsum.tile([128, 1024], f32, tag="slot")
                nc.tensor.matmul(
                    pj_ps[0:64, 0:512], wpj_sb[0:64],
                    ot_sb[:, 512 * h:512 * h + 512],
                    start=True, stop=True)
                nc.scalar.activation(y128[psl], pj_ps[0:64, 0:512], AF.Identity,
                                     bias=bnb_sb[psl], scale=bns_sb[psl])
                for p in range(4):
                    mr = masks[p].rearrange("p (i j) -> p i j", j=WP)
                    eng = nc.gpsimd if p == 3 else nc.vector
                    eng.tensor_tensor(
                        outr[psl, :, p // 2, :, p % 2], yr[psl], mr[psl],
                        op=OP.mult)
                nc.sync.dma_start(
                    out=out_d[:, 2048 * h:2048 * h + 2048],
                    in_=out_sb[64 * h:64 * h + 64, :])

    nc.compile()
    return nc


def _host_inputs(x, w_qkv, w_proj, gamma, beta, bn_mean, bn_var):
    """Build the per-core input maps (host-side packing)."""
    wq = w_qkv[:, 0:64]
    wk = w_qkv[:, 64:128]
    wv = np.ascontiguousarray(w_qkv[:, 128:192], dtype=np.float32)
    wb = np.zeros((128, _WBC), np.float32)
    for sg in range(4):
        for c in range(4):
            h = 4 * sg + c
            for d in range(HEAD_DIM):
                wb[0:64, _WQP0 + 128 * sg + 32 * c + d] = wq[:, 4 * h + d]
                wb[0:64, _WKP0 + 128 * sg + 32 * c + d] = wk[:, 4 * h + d]
    wb[0:64, _WV0:_WV0 + 64] = wv
    wb[0:64, _WPJ0:_WPJ0 + 64] = np.asarray(w_proj, dtype=np.float32)
    wb[0:64, _I0:_I0 + 128] = np.eye(64, 128, dtype=np.float32)
    wb[64:128, _I0:_I0 + 128] = np.eye(64, 128, 64, dtype=np.float32)
    inv = gamma / np.sqrt(bn_var + BN_EPS)
    wb[0:64, _BNS] = inv
    wb[0:64, _BNB] = beta - bn_mean * inv
    # double the 64-row blocks onto partitions 64..128 (identity handled above)
    wb[64:128, 0:_I0] = wb[0:64, 0:_I0]
    wb[64:128, _BNS:] = wb[0:64, _BNS:]

    shared = {"wb": wb}
    in_maps = []
    for b in range(B):
        m = dict(shared)
        m["x"] = np.ascontiguousarray(
            np.asarray(x)[b].reshape(DIM, H * W), dtype=np.float32)
        in_maps.append(m)
    return in_maps


def kernel(x, w_qkv, w_proj, gamma, beta, bn_mean, bn_var):
    from concourse import bass_utils

    if "nc" not in _CACHE:
        _CACHE["nc"] = _build_program()
    nc = _CACHE["nc"]
    in_maps = _host_inputs(
        np.asarray(x), np.asarray(w_qkv), np.asarray(w_proj),
        np.asarray(gamma), np.asarray(beta),
        np.asarray(bn_mean), np.asarray(bn_var))
    res = bass_utils.run_bass_kernel_spmd(nc, in_maps, core_ids=list(range(B)))
    out = np.stack([res.results[b]["out"].reshape(DIM, H, W) for b in range(B)])
    return out.astype(np.float32)


# revision 30
# speedup vs baseline: 1.0176x; 1.0176x over previous
"""Trainium2 Bass kernel for nn_D_GA_1812476199112 (maxpool -> 16-head
attention over 1024 tokens -> proj -> batchnorm -> maxunpool).

Sharding: data-parallel over batch B=8, one batch element per NeuronCore.
Everything is local per core; no collectives.

Per-core pipeline, v2 ("flip-AV + 2-engine exp"):
  * Pre/post stages run in a [128, 2048] partition-packed layout
    (partition = channel + 64*image-half), halving the per-op free sizes
    of maxpool / argmax masks / unpool and the x/out DMA times. All of
    them run on GPSIMD (which cannot touch PSUM), freeing ACT/DVE.
  * QKV prep matmuls run in f32r (1 cycle/row) with strip-packed
    host-permuted weights; Q/K packs are evicted PSUM->SBUF as paired
    [128, 1024] copies alternating ACT/DVE.
  * Scores S^T [128 keys, 1024 queries] per (head, kt) chunk (two 512-q
    matmuls into one 2-bank PSUM slot, 3 slots); exp is split across ACT
    (table exp, bf16 out) and DVE (Schraudolph bit-trick: int16
    y = round(a*s + b) reinterpreted as bf16 ~ exp(s/2), ~3% elementwise
    error that largely cancels in the softmax ratio; ~1e-3 end-to-end).
  * AV is "flipped": each E block [128k, 128q] is the stationary operand
    (weight load is free) against a ones-augmented V~ [128k, 5] moving
    operand, producing o^T [128q, 5] in 5 cycles/matmul, accumulated over
    key tiles into a memset 2-bank PSUM accumulator with start=False
    (a start=True would mark the whole 2KB zero-region pending-zero and
    wipe sibling regions); denominators ride along in column 4.
  * Tail: strided DVE reciprocal+normalize in [q, (h,d)] layout, identity
    matmul transpose back to [(h,d), q], f32r proj, fused BN via ACT
    Identity(scale,bias), unpool via masked multiplies on GPSIMD.
"""
import numpy as np

DIM = 64
HEAD_DIM = 4
NUM_HEADS = 16
B = 8
H = W = 64
HP = WP = 32
N = HP * WP          # 1024 tokens
NKT = 8              # key tiles of 128
BN_EPS = 1e-5

# Schraudolph constants for exp(0.5*s) in bf16-bits:
#   int16 y = round(A_SCH * s + B_SCH); reinterpret(y) ~ exp(0.5*s)
A_SCH = 0.5 * 128.0 / float(np.log(2.0))
B_SCH = 127.0 * 128.0 - 5.5

# wb column layout
_WQP0 = 0              # 4 x [128, 128] strip-packed q weights (doubled rows)
_WKP0 = 512            # 4 x [128, 128] strip-packed k weights
_WV0 = 1024            # [128, 64] v weights (doubled rows)
_WPJ0 = 1088           # [64, 64] proj weights (rows 0..64)
_I0 = 1152             # [128, 128] identity
_BNS = 1280            # [128, 1] bn scale (doubled)
_BNB = 1281            # [128, 1] bn bias (doubled)
_WBC = 1282

_CACHE = {}


def _region_col(m):
    """PSUM accumulator column for region m = 8*head + qtile (5 cols each,
    split so no [*,5] block crosses the 512-col PSUM bank boundary)."""
    return 5 * m if m < 102 else 512 + 5 * (m - 102)


def _exp_schedule():
    """Strict weighted interleave of the 128 exp chunks across ACT/DVE
    (Bresenham at the inverse-cost ratio) so the two engines always run
    concurrently; runs of the same engine serialize the 3-slot pipeline.
    The first few chunks go to ACT while DVE finishes its prep copies."""
    n_act_lead = 4
    frac_act = 0.5   # exact alternation avoids periodic double-ACT hiccups
    out = ["act"] * n_act_lead
    acc = 0.0
    for _ in range(128 - n_act_lead):
        acc += frac_act
        if acc >= 1.0:
            out.append("act")
            acc -= 1.0
        else:
            out.append("dve")
    return out


def _build_program():
    import concourse.mybir as mybir
    import concourse.tile as tile
    from concourse import bacc

    f32 = mybir.dt.float32
    f32r = mybir.dt.float32r
    bf16 = mybir.dt.bfloat16
    i16 = mybir.dt.int16
    AF = mybir.ActivationFunctionType
    OP = mybir.AluOpType

    nc = bacc.Bacc("TRN2", debug=False)

    x_d = nc.dram_tensor("x", [DIM, H * W], f32, kind="ExternalInput").ap()
    wb_d = nc.dram_tensor("wb", [128, _WBC], f32r, kind="ExternalInput").ap()
    out_d = nc.dram_tensor("out", [DIM, H * W], f32, kind="ExternalOutput").ap()

    with tile.TileContext(nc) as tc:
        with (
            tc.tile_pool(name="singles", bufs=1) as sg1,
            tc.tile_pool(name="expp", bufs=6) as expp,
            tc.tile_pool(name="spsum", bufs=3, space="PSUM") as spsum,
            tc.tile_pool(name="opsum", bufs=1, space="PSUM") as opsum,
        ):
            # hoist the ACT exp-table load to t=0 via a dummy exp
            warm = sg1.tile([1, 1], f32)
            nc.vector.memset(warm, 0.0)
            nc.scalar.activation(warm, warm, AF.Exp)
            ib16 = sg1.tile([128, 128], bf16)

            # ---------- input DMAs ----------
            x_sb = sg1.tile([128, H * W // 2], f32)
            nc.sync.dma_start(out=x_sb[0:64, :], in_=x_d[:, 0:2048])
            nc.sync.dma_start(out=x_sb[64:128, :], in_=x_d[:, 2048:4096])
            wb_sb = sg1.tile([128, _WBC], f32r)
            nc.sync.dma_start(out=wb_sb[:, 0:1024], in_=wb_d[:, 0:1024])
            nc.sync.dma_start(out=wb_sb[:, 1024:_WBC], in_=wb_d[:, 1024:_WBC])
            wqp = [wb_sb[:, _WQP0 + 128 * s:_WQP0 + 128 * s + 128] for s in range(4)]
            wkp = [wb_sb[:, _WKP0 + 128 * s:_WKP0 + 128 * s + 128] for s in range(4)]
            wv_sb = wb_sb[:, _WV0:_WV0 + 64]
            wpj_sb = wb_sb[:, _WPJ0:_WPJ0 + 64]
            i128_sb = wb_sb[:, _I0:_I0 + 128]
            bns_sb = wb_sb[:, _BNS:_BNS + 1].bitcast(f32)
            bnb_sb = wb_sb[:, _BNB:_BNB + 1].bitcast(f32)

            # ---------- PE warmup (p-state ramp) ----------
            dummy_bf = sg1.tile([64, 512], bf16)
            nc.vector.memset(dummy_bf, 1.0)
            for _ in range(11):
                w_ps = spsum.tile([128, 1024], f32, tag="slot")
                nc.tensor.matmul(w_ps[:, 0:512], dummy_bf[:, 0:128], dummy_bf,
                                 start=True, stop=True)

            # ---------- maxpool (DVE max; GPSIMD lacks max/is_equal) ----------
            # x_sb per-partition layout: 32 h-rows x 64 w; window elems:
            xr = x_sb.rearrange("p (i ti j tj) -> p i ti j tj", ti=2, tj=2, j=WP)
            v = [xr[:, :, 0, :, 0], xr[:, :, 0, :, 1],
                 xr[:, :, 1, :, 0], xr[:, :, 1, :, 1]]
            m01 = sg1.tile([128, N // 2], f32)
            m23 = sg1.tile([128, N // 2], f32)
            m01r = m01.rearrange("p (i j) -> p i j", j=WP)
            m23r = m23.rearrange("p (i j) -> p i j", j=WP)
            pooled = sg1.tile([128, N // 2], f32)
            pooledfr = pooled.rearrange("p (i j) -> p i j", j=WP)
            pooled2 = sg1.tile([128, N // 2], f32r)
            pooled2r = pooled2.rearrange("p (i j) -> p i j", j=WP)
            nc.vector.tensor_tensor(m01r, v[0], v[1], op=OP.max)
            nc.vector.tensor_tensor(m23r, v[2], v[3], op=OP.max)
            # f32r first (gates prep); exact f32 after (gates only masks)
            nc.vector.tensor_tensor(pooled2r, m01r, m23r, op=OP.max)
            nc.vector.tensor_tensor(pooledfr, m01r, m23r, op=OP.max)

            # ---------- argmax masks, entirely on idle GPSIMD ----------
            # eq via saturating int16 trick: d = pooled - v >= 0;
            # int16(-3.4e38*d + 16256) = 16256 (bf16 1.0) iff d == 0, else
            # saturates to -32768 (bf16 -0.0). First-match chain as usual.
            masks = []
            nf = None
            for p in range(4):
                dterm = sg1.tile([128, N // 2], f32, tag=f"dt{p}")
                nc.gpsimd.tensor_tensor(
                    dterm.rearrange("p (i j) -> p i j", j=WP), pooledfr, v[p],
                    op=OP.subtract)
                eqi = sg1.tile([128, N // 2], i16, tag=f"eqi{p}")
                nc.gpsimd.tensor_scalar(eqi, dterm, -3.4e38, 16256.0,
                                        op0=OP.mult, op1=OP.add)
                eq = eqi[:, :].bitcast(bf16)
                if p == 0:
                    nf = sg1.tile([128, N // 2], f32, tag="nf0")
                    nc.gpsimd.tensor_scalar(nf, eq, -1.0, 1.0,
                                            op0=OP.mult, op1=OP.add)
                    mk0 = sg1.tile([128, N // 2], f32, tag="mk0")
                    nc.gpsimd.tensor_scalar(mk0, nf, -1.0, 1.0,
                                            op0=OP.mult, op1=OP.add)
                    masks.append(mk0[:, :])
                else:
                    mk = sg1.tile([128, N // 2], f32, tag=f"mk{p}")
                    nc.gpsimd.tensor_tensor(mk, eq, nf, op=OP.mult)
                    masks.append(mk[:, :])
                    if p < 3:
                        nf2 = sg1.tile([128, N // 2], f32, tag=f"nf{p}")
                        nc.gpsimd.tensor_tensor(nf2, nf, mk, op=OP.subtract)
                        nf = nf2

            # ---------- QKV prep (f32r matmuls; paired ACT/DVE evictions) ----
            qtp = [sg1.tile([128, N], f32r, tag=f"qtp{s}", name=f"qtp{s}")
                   for s in range(4)]
            ktp = [sg1.tile([128, N], f32r, tag=f"ktp{s}", name=f"ktp{s}")
                   for s in range(4)]
            pooled_r = pooled2[:, :]

            evict = [nc.scalar.copy, nc.vector.tensor_copy]
            ev = 0
            for s in range(4):
                for w_sb, pack in ((wkp[s], ktp[s]), (wqp[s], qtp[s])):
                    p_ps = spsum.tile([128, 1024], f32, tag="slot",
                                      name="p_ps")
                    for h in range(2):
                        psl = slice(64 * h, 64 * h + 64)
                        nc.tensor.matmul(p_ps[:, 512 * h:512 * h + 512],
                                         w_sb[psl],
                                         pooled_r[psl], start=True, stop=True)
                    evict[ev % 2](pack[:, :], p_ps)
                    ev += 1

            # V~ tiles [128 keys, 16 heads * (4 dims + ones)] in bf16;
            # the per-kt prep matmul+copy is emitted inside the chunk loop
            # (at sg==0) so it doesn't delay the first score chunks.
            vt = [sg1.tile([128, 80], bf16, tag=f"vt{kt}", name=f"vt{kt}")
                  for kt in range(NKT)]
            for kt in range(NKT):
                nc.gpsimd.memset(vt[kt], 1.0)

            def emit_vprep(kt):
                h = kt // 4
                psl = slice(64 * h, 64 * h + 64)
                v_ps = spsum.tile([128, 1024], f32, tag="slot", name="v_ps")
                nc.tensor.matmul(
                    v_ps[:, 0:64],
                    pooled_r[psl, 128 * (kt % 4):128 * (kt % 4) + 128],
                    wv_sb[psl], start=True, stop=True)
                vcp = nc.scalar.copy if kt % 2 == 0 else nc.vector.tensor_copy
                vcp(vt[kt].rearrange("p (h e) -> p h e", e=5)[:, :, 0:4],
                    v_ps[:, 0:64].rearrange("p (h e) -> p h e", e=4))

            # ---------- attention ----------
            # Bank-init trick: one start=True matmul into each bank's pad
            # columns marks the whole 2KB zero-region pending-zero, so every
            # region's first start=False AV write initializes (no memset).
            # More than one start=True per bank would wipe sibling regions.
            o_acc = opsum.tile([128, 1024], f32)
            for bank_pad in (510, 1022):
                nc.tensor.matmul(o_acc[:, bank_pad:bank_pad + 2],
                                 dummy_bf[:, 0:128], dummy_bf[:, 0:2],
                                 start=True, stop=True,
                                 skip_group_check=True)
            assign = _exp_schedule()
            pend = []
            LAG = 3

            def flush_one():
                e_t, sg, c, kt = pend.pop(0)
                head = 4 * sg + c
                for J in range(8):
                    col = _region_col(16 * J + head)
                    nc.tensor.matmul(
                        o_acc[:, col:col + 5],
                        e_t[:, 128 * J:128 * J + 128].bitcast(bf16),
                        vt[kt][:, 5 * head:5 * head + 5],
                        start=False, stop=(kt == NKT - 1),
                        skip_group_check=True)

            ci = 0
            for sg in range(4):
                for kt in range(NKT):
                    for c in range(4):
                        s_ps = spsum.tile([128, 1024], f32, tag="slot")
                        for qh in range(2):
                            nc.tensor.matmul(
                                s_ps[:, 512 * qh:512 * qh + 512],
                                ktp[sg][32 * c:32 * c + 4,
                                        128 * kt:128 * kt + 128],
                                qtp[sg][32 * c:32 * c + 4,
                                        512 * qh:512 * qh + 512],
                                start=True, stop=True,
                                tile_position=(32 * c, 0))
                        e_t = expp.tile([128, 1024], i16, tag="exp")
                        eng = assign[ci]
                        ci += 1
                        if eng == "act":
                            nc.scalar.activation(
                                e_t[:, :].bitcast(bf16), s_ps,
                                AF.Exp, scale=0.5)
                        else:
                            nc.vector.tensor_scalar(
                                e_t, s_ps, A_SCH, B_SCH,
                                op0=OP.mult, op1=OP.add)
                        pend.append((e_t, sg, c, kt))
                        if ci <= NKT:
                            # spread the V~ preps over the first chunks
                            emit_vprep(ci - 1)
                        if len(pend) > LAG:
                            flush_one()
            while pend:
                flush_one()

            # ---------- tail ----------
            # denominators: region m col +4, strided over the 5-col regions
            o5a = o_acc[:, 0:510].rearrange("p (m e) -> p m e", e=5)
            o5b = o_acc[:, 512:642].rearrange("p (m e) -> p m e", e=5)
            rall = sg1.tile([128, 128], f32)
            nc.vector.reciprocal(rall[:, 0:102], o5a[:, :, 4])
            nc.vector.reciprocal(rall[:, 102:128], o5b[:, :, 4])
            rep = sg1.tile([128, 512], f32)
            rep4 = rep.rearrange("p (m e) -> p m e", e=4)
            for d in range(4):
                nc.vector.tensor_copy(rep4[:, :, d], rall)
            eno = sg1.tile([128, 512], bf16)
            eno4 = eno.rearrange("p (m e) -> p m e", e=4)
            nc.vector.tensor_tensor(eno4[:, 0:102], o5a[:, :, 0:4],
                                    rep4[:, 0:102], op=OP.mult)
            nc.vector.tensor_tensor(eno4[:, 102:128], o5b[:, :, 0:4],
                                    rep4[:, 102:128], op=OP.mult)
            # transpose back to [(h,d), q] via identity matmuls; eno cols are
            # (J, h, d)-contiguous under the J-major region map m = 16J+head
            nc.scalar.copy(ib16, i128_sb)
            ot_sb = sg1.tile([64, N], f32r)
            for jg in range(2):
                ot_ps = spsum.tile([128, 1024], f32, tag="slot")
                for jj in range(4):
                    J = 4 * jg + jj
                    nc.tensor.matmul(
                        ot_ps[0:64, 128 * jj:128 * jj + 128],
                        eno[:, 64 * J:64 * J + 64],
                        ib16,
                        start=True, stop=True)
                otcp = nc.scalar.copy if jg == 0 else nc.vector.tensor_copy
                otcp(ot_sb[:, 512 * jg:512 * jg + 512], ot_ps[0:64, 0:512])
            # proj + BN (per image-half) + unpool
            y128 = sg1.tile([128, 512], f32)
            out_sb = sg1.tile([128, H * W // 2], f32)
            outr = out_sb.rearrange("p (i ti j tj) -> p i ti j tj",
                                    ti=2, tj=2, j=WP)
            yr = y128.rearrange("p (i j) -> p i j", j=WP)
            for h in range(2):
                psl = slice(64 * h, 64 * h + 64)
                pj_ps = spsum.tile([128, 1024], f32, tag="slot")
                nc.tensor.matmul(
                    pj_ps[0:64, 0:512], wpj_sb[0:64],
                    ot_sb[:, 512 * h:512 * h + 512],
                    start=True, stop=True)
                nc.scalar.activation(y128[psl], pj_ps[0:64, 0:512], AF.Identity,
                                     bias=bnb_sb[psl], scale=bns_sb[psl])
                for p in range(4):
                    mr = masks[p].rearrange("p (i j) -> p i j", j=WP)
                    eng = nc.gpsimd if p == 3 else nc.vector
                    eng.tensor_tensor(
                        outr[psl, :, p // 2, :, p % 2], yr[psl], mr[psl],
                        op=OP.mult)
                nc.sync.dma_start(
                    out=out_d[:, 2048 * h:2048 * h + 2048],
                    in_=out_sb[64 * h:64 * h + 64, :])

    nc.compile()
    return nc


def _host_inputs(x, w_qkv, w_proj, gamma, beta, bn_mean, bn_var):
    """Build the per-core input maps (host-side packing)."""
    wq = w_qkv[:, 0:64]
    wk = w_qkv[:, 64:128]
    wv = np.ascontiguousarray(w_qkv[:, 128:192], dtype=np.float32)
    wb = np.zeros((128, _WBC), np.float32)
    for sg in range(4):
        for c in range(4):
            h = 4 * sg + c
            for d in range(HEAD_DIM):
                wb[0:64, _WQP0 + 128 * sg + 32 * c + d] = wq[:, 4 * h + d]
                wb[0:64, _WKP0 + 128 * sg + 32 * c + d] = wk[:, 4 * h + d]
    wb[0:64, _WV0:_WV0 + 64] = wv
    wb[0:64, _WPJ0:_WPJ0 + 64] = np.asarray(w_proj, dtype=np.float32)
    wb[0:64, _I0:_I0 + 128] = np.eye(64, 128, dtype=np.float32)
    wb[64:128, _I0:_I0 + 128] = np.eye(64, 128, 64, dtype=np.float32)
    inv = gamma / np.sqrt(bn_var + BN_EPS)
    wb[0:64, _BNS] = inv
    wb[0:64, _BNB] = beta - bn_mean * inv
    # double the 64-row blocks onto partitions 64..128 (identity handled above)
    wb[64:128, 0:_I0] = wb[0:64, 0:_I0]
    wb[64:128, _BNS:] = wb[0:64, _BNS:]

    shared = {"wb": wb}
    in_maps = []
    for b in range(B):
        m = dict(shared)
        m["x"] = np.ascontiguousarray(
            np.asarray(x)[b].reshape(DIM, H * W), dtype=np.float32)
        in_maps.append(m)
    return in_maps


def kernel(x, w_qkv, w_proj, gamma, beta, bn_mean, bn_var):
    from concourse import bass_utils

    if "nc" not in _CACHE:
        _CACHE["nc"] = _build_program()
    nc = _CACHE["nc"]
    in_maps = _host_inputs(
        np.asarray(x), np.asarray(w_qkv), np.asarray(w_proj),
        np.asarray(gamma), np.asarray(beta),
        np.asarray(bn_mean), np.asarray(bn_var))
    res = bass_utils.run_bass_kernel_spmd(nc, in_maps, core_ids=list(range(B)))
    out = np.stack([res.results[b]["out"].reshape(DIM, H, W) for b in range(B)])
    return out.astype(np.float32)
